# revision 16
# baseline (speedup 1.0000x reference)
"""Tensor-parallel causal GQA self-attention (B=1, S=2048, D=4096, 32 q heads /
8 kv heads, HD=128, interleaved RoPE) on 8 trn2 NeuronCores.

Sharding: core c owns kv head c and q heads 4c..4c+3 (column-parallel
Wq/Wk/Wv, row-parallel Wo).  Each core computes a full [S, D] partial of the
output projection; the host sums the 8 partials (the "all-reduce").

Speed strategy (v2): fp8e4 (e4m3) matmuls with MatmulPerfMode.DoubleRow pack
TWO k-tiles per pass at 0.5 cycles/row -- 4x the fp32r rate -- for the QKV
projections and o_proj.  Attention core (scores/AV) stays fp32r for now.
fp8's ~2.4% RMS quantization error is fine for most rows (softmax averaging
scales output magnitude ~1/sqrt(row)), but rows 0..127 -- which dominate
the output absmax -- are recomputed in an fp16 "patch" pass and overwritten
host-side.

Scaling scheme (keeps fp8 operands ~N(0,1)):
  Wq8/Wk8/Wv8 = fp8(W * 64)      -> q,k,v come out of PSUM x64
  rope cos/sin tables x(1/64)    -> QT/KT at true scale
  V copy applies scale 1/64      -> V at true scale
  softmax 1/sqrt(HD) applied as the Exp activation's scale argument
  otn8 = 16/denom * OT (fp8)     -> Wo8 = fp8(Wo * 64); out copies x(1/1024)
  patch path: Wqkv16 = W*64 (same rope/V handling), Wo16 unscaled.
"""

import sys

if "/opt/trn_rl_repo" not in sys.path:
    sys.path.insert(0, "/opt/trn_rl_repo")

import numpy as np
import ml_dtypes

import concourse.bass as bass
import concourse.tile as tile
from concourse import bacc, mybir
from concourse.bass_utils import run_bass_kernel_spmd

S, D, NH, NKV, HD = 2048, 4096, 32, 8, 128
NCORES = 8
QH = NH // NCORES  # 4 q heads per core
ROPE_BASE = 500000.0

F32 = mybir.dt.float32
F32R = mybir.dt.float32r
F16 = mybir.dt.float16
BF16 = mybir.dt.bfloat16
F8 = mybir.dt.float8e4
AF = mybir.ActivationFunctionType
DR = mybir.MatmulPerfMode.DoubleRow
FP8NP = ml_dtypes.float8_e4m3
BF16NP = ml_dtypes.bfloat16

SC = S // 512   # 4 s-chunks of 512
DP = D // 256   # 16 d-tile pairs
DG = DP // 2    # 8 dma groups (two pairs per DMA)
JT = S // 128   # 16 j-tiles of 128

EXP_SCALE = float(1.0 / np.sqrt(HD))
EXP_BIAS = float(-np.log(16.0))  # keep exp() outputs in fp8 range
OSC = 1.0 / 1024.0  # undo otn x16 and Wo x64

_CACHE = {}

# set by test harness to collect an exec-time profile
TRACE = False
LAST_EXEC_NS = None


def _build_nc():
    nc = bacc.Bacc("TRN2", target_bir_lowering=False, debug=False,
                   num_devices=NCORES)

    xw_d = nc.declare_dram_parameter("xw8", [SC, DG, 128, 3072], F8,
                                     isOutput=False)
    wq_d = nc.declare_dram_parameter("wq8", [DP, 128, 1024], F8,
                                     isOutput=False)
    wo_d = nc.declare_dram_parameter("wo8", [8, 2, 128, 1024], F8,
                                     isOutput=False)
    cos_d = nc.declare_dram_parameter("cos2", [SC, HD, 512], F32, isOutput=False)
    sin_d = nc.declare_dram_parameter("sin2", [SC, HD, 512], F32, isOutput=False)
    perm_d = nc.declare_dram_parameter("perm", [HD, HD], F32R, isOutput=False)
    masks_d = nc.declare_dram_parameter("masks", [128, 512], F32,
                                        isOutput=False)
    negI_d = nc.declare_dram_parameter("negI", [128, 128], F32R,
                                       isOutput=False)
    sl_d = nc.declare_dram_parameter("sl", [4, 128, 512], F32R,
                                     isOutput=False)
    on16_d = nc.declare_dram_parameter("on16", [128, 256], F8,
                                       isOutput=False)
    onr_d = nc.declare_dram_parameter("ones_red", [128, 1], F32R,
                                      isOutput=False)
    onb_d = nc.declare_dram_parameter("onb16", [1, 128], F32R,
                                      isOutput=False)
    onb1_d = nc.declare_dram_parameter("onb1", [1, 128], F32R,
                                       isOutput=False)
    onr16_d = nc.declare_dram_parameter("onr16", [128, 1], F16,
                                        isOutput=False)
    ident_d = nc.declare_dram_parameter("ident", [HD, HD], F32R,
                                        isOutput=False)
    x16_d = nc.declare_dram_parameter("x16", [128, 4096], F16, isOutput=False)
    wqkv16_d = nc.declare_dram_parameter("wqkv16", [128, 32, 768], F16,
                                         isOutput=False)
    wo16_d = nc.declare_dram_parameter("wo16", [128, 4, 4096], F16,
                                       isOutput=False)
    out_d = nc.declare_dram_parameter("out", [8, 4, 128, 2048], BF16,
                                      isOutput=True)
    outp_d = nc.declare_dram_parameter("outp", [2, 128, 2048], BF16,
                                       isOutput=True)

    with tile.TileContext(nc) as tc:
        from contextlib import ExitStack
        ctx = ExitStack()
        with ctx:
            wpool = ctx.enter_context(tc.tile_pool(name="wpool", bufs=16))
            wopool = ctx.enter_context(tc.tile_pool(name="wopool", bufs=16))
            xpool = ctx.enter_context(tc.tile_pool(name="xpool", bufs=3))
            qtp = ctx.enter_context(tc.tile_pool(name="qtp", bufs=9))
            otnp = ctx.enter_context(tc.tile_pool(name="otnp", bufs=8))
            tabp = ctx.enter_context(tc.tile_pool(name="tabp", bufs=4))
            ktp = ctx.enter_context(tc.tile_pool(name="ktp", bufs=4))
            vnp = ctx.enter_context(tc.tile_pool(name="vnp", bufs=4))
            stg = ctx.enter_context(tc.tile_pool(name="stg", bufs=4))
            rawp = ctx.enter_context(tc.tile_pool(name="rawp", bufs=6))
            ptp = ctx.enter_context(tc.tile_pool(name="ptp", bufs=4))
            mkp = ctx.enter_context(tc.tile_pool(name="mkp", bufs=4))
            cst = ctx.enter_context(tc.tile_pool(name="cst", bufs=1))
            rcp = ctx.enter_context(tc.tile_pool(name="rcp", bufs=1))
            ostp = ctx.enter_context(tc.tile_pool(name="ostp", bufs=3))
            # patch pools
            p16 = ctx.enter_context(tc.tile_pool(name="p16", bufs=1))
            pw16 = ctx.enter_context(tc.tile_pool(name="pw16", bufs=16))
            pwo16 = ctx.enter_context(tc.tile_pool(name="pwo16", bufs=6))
            pst = ctx.enter_context(tc.tile_pool(name="pst", bufs=4))
            psA = ctx.enter_context(
                tc.tile_pool(name="psA", bufs=6, space=bass.MemorySpace.PSUM))
            psB = ctx.enter_context(
                tc.tile_pool(name="psB", bufs=2, space=bass.MemorySpace.PSUM))

            # small tables, spread DMAs across both issue paths
            perm_t = cst.tile([HD, HD], F32R, name="perm_t")
            ident_t = cst.tile([HD, HD], F32R, name="ident_t")
            onr_t = cst.tile([128, 1], F32R, name="onr_t")
            onb_t = cst.tile([1, 128], F32R, name="onb_t")
            onb1_t = cst.tile([1, 128], F32R, name="onb1_t")
            onr16_t = cst.tile([128, 1], F16, name="onr16_t")
            mask0_t = mkp.tile([128, 512], F32, name="mask0_t")
            negI_t = mkp.tile([128, 128], F32R, name="negI_t")
            sl_t = [mkp.tile([128, 512], F32R, tag="sl", name=f"sl_{m}")
                    for m in range(4)]
            on16_t = cst.tile([128, 256], F8, name="on16_t")
            ebias_t = cst.tile([128, 1], F32, name="ebias_t")
            nc.gpsimd.memset(ebias_t[:], EXP_BIAS)

            def table_loads():
                yield lambda: nc.sync.dma_start(perm_t[:], perm_d[:])
                yield lambda: nc.scalar.dma_start(ident_t[:], ident_d[:])
                yield lambda: nc.sync.dma_start(onr_t[:], onr_d[:])
                yield lambda: nc.scalar.dma_start(onb_t[:], onb_d[:])
                yield lambda: nc.scalar.dma_start(onb1_t[:], onb1_d[:])
                yield lambda: nc.sync.dma_start(onr16_t[:], onr16_d[:])
                yield lambda: nc.scalar.dma_start(mask0_t[:], masks_d[:])
                yield lambda: nc.sync.dma_start(negI_t[:], negI_d[:])
                yield lambda: nc.scalar.dma_start(on16_t[:], on16_d[:])
                for m in range(4):
                    eng = nc.sync if m % 2 == 0 else nc.scalar
                    yield lambda m=m, eng=eng: eng.dma_start(
                        sl_t[m][:], sl_d[m])

            wq_tiles = [None] * DP

            # persistent activations, one tile per (tensor, s-chunk)
            QTc = [[qtp.tile([HD, 512], F32R, tag="qtc", name=f"qt{h}_{c}")
                    for c in range(SC)] for h in range(QH)]
            KTc = [ktp.tile([HD, 512], F32R, tag="ktc", name=f"kt{c}")
                   for c in range(SC)]
            Vnc = [vnp.tile([128, 512], F8, tag="vnc", name=f"vn{c}")
                   for c in range(SC)]
            # otn8[c][hp]: [128, 1024] fp8 = [hd, (head 2*hp) 512 i | (2*hp+1)]
            otn8 = [[otnp.tile([128, 1024], F8, tag="otn", name=f"otn{c}_{p}")
                     for p in range(2)] for c in range(SC)]

            # ---- phase 1: QKV projections + RoPE + V transpose ----
            def rope_copy(acc_ps, eng):
                raw = rawp.tile([128, 512], F32R, tag="raw", name="rope_raw")
                if eng == "act":
                    nc.scalar.activation(raw[:], acc_ps[:], AF.Copy)
                else:
                    nc.vector.tensor_copy(raw[:], acc_ps[:])
                return raw

            def rope_combine(raw, dest, cc, sn):
                rot = psB.tile([128, 512], F32, tag="tmp", name="rope_rot")
                nc.tensor.matmul(rot[:], perm_t[:], raw[:], start=True,
                                 stop=True)
                t1 = stg.tile([128, 512], F32, tag="stg", name="rope_t1")
                nc.gpsimd.tensor_mul(t1[:], raw[:].bitcast(F32), cc[:])
                t2 = stg.tile([128, 512], F32, tag="stg", name="rope_t2")
                nc.vector.tensor_mul(t2[:], rot[:], sn[:])
                nc.gpsimd.tensor_add(dest[:], t1[:], t2[:])

            boundary_pe = [None]

            def emit_boundary_pe():
                if boundary_pe[0] is not None:
                    boundary_pe[0]()
                    boundary_pe[0] = None

            cs_tiles = [None] * SC  # (cos, sin) chunk tiles, single-use

            # ---- phase 2: attention, interleaved with QKV by chunk ----
            tails = []

            def make_tail(c, h, ot, dsum_bc):
                def tail():
                    rcb = stg.tile([128, 512], F32, tag="stg", name="rcb")
                    with nc.allow_low_precision(reason="fp22 softmax recip"):
                        nc.vector.reciprocal(rcb[:], dsum_bc[:])
                    dst = otn8[c][h // 2][:, 512 * (h % 2):512 * (h % 2 + 1)]
                    nc.vector.tensor_mul(dst, ot[:], rcb[:])
                return tail

            wo_tiles = []

            def o_proj_chunk(c):
                late = c == SC - 1
                for dc in range(8):
                    woa, wob = wo_tiles[dc]
                    ost = ostp.tile([128, 2048], BF16, tag="ost",
                                    name="ost")
                    for kk in range(4):
                        acc = psA.tile([128, 512], F32, tag="acc",
                                       name="oacc")
                        for hp in range(2):
                            lhs = otn8[c][hp][:].rearrange(
                                "p (a b) -> p a b",
                                b=512)[:, :, 128 * kk:128 * (kk + 1)]
                            rhs = woa if hp == 0 else wob
                            nc.tensor.matmul(acc[:], lhs, rhs[:],
                                             start=(hp == 0),
                                             stop=(hp == 1),
                                             perf_mode=DR)
                        dstc = ost[:, 512 * kk:512 * (kk + 1)]
                        act_take = kk == 0 or (late and kk == 1)
                        if act_take:
                            nc.scalar.mul(dstc, acc[:], OSC)
                        else:
                            nc.vector.tensor_scalar_mul(dstc, acc[:], OSC)
                    eng = nc.sync if dc % 2 == 0 else nc.scalar
                    eng.dma_start(out_d[dc, c], ost[:])

            def attn_chunk(c):
                for h in range(QH):
                    qch = QTc[h][c][:]
                    ot = psA.tile([128, 512], F32, tag="acc", name="ot_ps")
                    dsum_bc = psB.tile([128, 512], F32, tag="tmp",
                                       name="dsum_bc")
                    npair = 2 * c + 2
                    pts = [None] * npair

                    def score_pair(t, c=c, qch=qch):
                        pt8 = ptp.tile([128, 1024], F8, tag="pt", name="pt8")
                        for u in range(2):
                            jt = 2 * t + u
                            stp = psA.tile([128, 512], F32, tag="acc",
                                           name="stp")
                            m = jt - 4 * c
                            nc.tensor.matmul(
                                stp[:],
                                KTc[jt // 4][:, 128 * (jt % 4):
                                             128 * (jt % 4 + 1)],
                                qch, start=True, stop=(m < 0))
                            if m >= 0:
                                # additive -1e30 causal mask in PSUM
                                nc.tensor.matmul(stp[:], negI_t[:],
                                                 sl_t[m][:],
                                                 start=False, stop=True)
                            nc.scalar.activation(
                                pt8[:, 512 * u:512 * (u + 1)], stp[:],
                                AF.Exp, scale=EXP_SCALE, bias=ebias_t[:])
                        return pt8

                    def accum_pair(t, pt8, ot=ot, dsum_bc=dsum_bc,
                                   npair=npair):
                        v4 = Vnc[t // 2][:].rearrange("p (a b) -> p a b",
                                                      b=128)
                        pr = pt8[:].rearrange("p (a b) -> p a b", b=512)
                        nc.tensor.matmul(
                            ot[:], v4[:, 2 * (t % 2):2 * (t % 2) + 2, :],
                            pr, start=(t == 0), stop=(t == npair - 1),
                            perf_mode=DR)
                        # all-(1/16) weights: every output row accumulates
                        # sum_j P/256 -> denominator pre-broadcast to all
                        # 128 partitions (x16 otn scale folded in)
                        nc.tensor.matmul(
                            dsum_bc[:],
                            on16_t[:].rearrange("p (a b) -> p a b", b=128),
                            pr, start=(t == 0), stop=(t == npair - 1),
                            perf_mode=DR)

                    for t in range(npair):
                        pts[t] = score_pair(t)
                        if t >= 1:
                            accum_pair(t - 1, pts[t - 1])
                    accum_pair(npair - 1, pts[npair - 1])
                    tails.append(make_tail(c, h, ot, dsum_bc))
                    if len(tails) > 1:
                        tails.pop(0)()
                while tails:
                    tails.pop(0)()
                o_proj_chunk(c)

            pload = {}

            def patch_loads():
                x16t = p16.tile([128, 4096], F16, name="x16t")
                nc.gpsimd.dma_start(x16t[:], x16_d[:])
                pcc = p16.tile([128, 128], F32, name="pcc")
                nc.gpsimd.dma_start(pcc[:], cos_d[0, :, 0:128])
                psn = p16.tile([128, 128], F32, name="psn")
                nc.gpsimd.dma_start(psn[:], sin_d[0, :, 0:128])
                wts = []
                for kt in range(32):
                    wt16 = pw16.tile([128, 768], F16, tag="pw",
                                     name="wt16")
                    nc.gpsimd.dma_start(wt16[:], wqkv16_d[:, kt, :])
                    wts.append(wt16)
                pload.update(x16t=x16t, pcc=pcc, psn=psn, wts=wts)

            for sc in range(SC):
                qps = [psA.tile([128, 512], F32, tag="acc", name=f"qps{h}")
                       for h in range(QH)]
                kps = psA.tile([128, 512], F32, tag="acc", name="kps")
                vps = psA.tile([128, 512], F32, tag="acc", name="vps")
                for dg in range(DG):
                    if sc == 0:
                        for half in range(2):
                            wt = wpool.tile([128, 2, 512], F8, tag="w",
                                            name=f"wq{2 * dg + half}")
                            nc.scalar.dma_start(wt[:], wq_d[2 * dg + half])
                            wq_tiles[2 * dg + half] = wt
                    xt = xpool.tile([128, 3072], F8, tag="x", name="xt")
                    xeng = nc.sync if dg % 2 == 0 else nc.gpsimd
                    xeng.dma_start(xt[:], xw_d[sc, dg])
                    if sc == 0:
                        if dg == 0:
                            _tl = table_loads()
                        next(_tl, lambda: None)()
                        next(_tl, lambda: None)()
                    if dg == 1:
                        emit_boundary_pe()
                    if dg == 4:
                        # prefetch this boundary's cos/sin chunk
                        cc = tabp.tile([128, 512], F32, tag="tab", name="cc")
                        nc.sync.dma_start(cc[:], cos_d[sc])
                        sn = tabp.tile([128, 512], F32, tag="tab", name="sn")
                        nc.sync.dma_start(sn[:], sin_d[sc])
                        cs_tiles[sc] = (cc, sn)
                    x6 = xt[:].rearrange("p (a b) -> p a b", b=512)
                    x24 = xt[:].rearrange("p (a b) -> p a b", b=128)
                    for g in range(2):
                        dp = 2 * dg + g
                        xv = x6[:, 3 * g:3 * g + 2, :]
                        kw = x24[:, 12 * g + 8:12 * g + 10, :]
                        vw = x24[:, 12 * g + 10:12 * g + 12, :]
                        st = dp == 0
                        sp = dp == DP - 1
                        wqt = wq_tiles[dp]
                        for h in range(QH):
                            nc.tensor.matmul(
                                qps[h][:], wqt[:, :, HD * h:HD * (h + 1)],
                                xv, start=st, stop=sp, perf_mode=DR)
                        nc.tensor.matmul(kps[:], kw, xv, start=st, stop=sp,
                                         perf_mode=DR)
                        nc.tensor.matmul(vps[:], vw, xv, start=st, stop=sp,
                                         perf_mode=DR)

                raw_k = rope_copy(kps, "act")
                vt_sb = rope_copy(vps, "dve")
                raw_q = [None] * QH
                raw_q[0] = rope_copy(qps[0], "dve")

                def boundary(sc=sc, raw_k=raw_k, vt_sb=vt_sb, raw_q=raw_q):
                    cc, sn = cs_tiles[sc]
                    rope_combine(raw_k, KTc[sc], cc, sn)
                    rope_combine(raw_q[0], QTc[0][sc], cc, sn)
                    for k4 in range(4):
                        vtp = psB.tile([128, 128], F32R, tag="tmp",
                                       name="vtp")
                        nc.tensor.transpose(
                            vtp[:], vt_sb[:, 128 * k4:128 * (k4 + 1)],
                            ident_t[:])
                        nc.scalar.activation(
                            Vnc[sc][:, 128 * k4:128 * (k4 + 1)],
                            vtp[:].bitcast(F32), AF.Copy, scale=1.0 / 64.0)
                    for h in range(1, QH):
                        rope_combine(raw_q[h], QTc[h][sc], cc, sn)

                boundary_pe[0] = boundary
                if sc == 1:
                    for dc in range(8):
                        woa = wopool.tile([128, 2, 512], F8, tag="wo",
                                          name=f"woa{dc}")
                        nc.gpsimd.dma_start(woa[:], wo_d[dc, 0])
                        wob = wopool.tile([128, 2, 512], F8, tag="wo",
                                          name=f"wob{dc}")
                        nc.gpsimd.dma_start(wob[:], wo_d[dc, 1])
                        wo_tiles.append((woa, wob))
                if sc >= 1:
                    attn_chunk(sc - 1)
                for h in range(1, QH):
                    raw_q[h] = rope_copy(qps[h],
                                         "act" if h % 2 == 0 else "dve")
                if sc == SC - 1:
                    emit_boundary_pe()
                    attn_chunk(SC - 1)
                    patch_loads()

            # ---- phase 4: fp16 patch for output rows 0..127 ----
            def patch():
                x16t, pcc, psn = pload["x16t"], pload["pcc"], pload["psn"]
                pq = [psA.tile([128, 128], F32, tag="acc", name=f"pq{h}")
                      for h in range(QH)]
                pk = psA.tile([128, 128], F32, tag="acc", name="pk")
                pv = psA.tile([128, 128], F32, tag="acc", name="pv")
                for kt in range(32):
                    wt16 = pload["wts"][kt]
                    rhs = x16t[:, 128 * kt:128 * (kt + 1)]
                    st = kt == 0
                    sp = kt == 31
                    for h in range(QH):
                        nc.tensor.matmul(pq[h][:],
                                         wt16[:, 128 * h:128 * (h + 1)],
                                         rhs, start=st, stop=sp)
                    nc.tensor.matmul(pk[:], wt16[:, 512:640], rhs,
                                     start=st, stop=sp)
                    nc.tensor.matmul(pv[:], wt16[:, 640:768], rhs,
                                     start=st, stop=sp)

                # patch rope (fp32 math, fp16 outputs)
                def prope(acc_ps, nm):
                    raw = pst.tile([128, 128], F32R, tag="pst",
                                   name=f"raw_{nm}")
                    nc.scalar.activation(raw[:], acc_ps[:], AF.Copy)
                    rot = psB.tile([128, 128], F32, tag="tmp", name="prot")
                    nc.tensor.matmul(rot[:], perm_t[:], raw[:], start=True,
                                     stop=True)
                    t1 = pst.tile([128, 128], F32, tag="pst", name="pt1")
                    nc.vector.tensor_mul(t1[:], raw[:].bitcast(F32), pcc[:])
                    t2 = pst.tile([128, 128], F32, tag="pst", name="pt2")
                    nc.vector.tensor_mul(t2[:], rot[:], psn[:])
                    d16 = p16.tile([128, 128], F16, name=f"d16_{nm}")
                    nc.vector.tensor_add(d16[:], t1[:], t2[:])
                    return d16

                KT16 = prope(pk, "k")
                QT16 = [prope(pq[h], f"q{h}") for h in range(QH)]
                pv_sb = pst.tile([128, 128], F32R, tag="pst", name="pv_sb")
                nc.vector.tensor_copy(pv_sb[:], pv[:])
                vtp = psB.tile([128, 128], F32R, tag="tmp", name="pvtp")
                nc.tensor.transpose(vtp[:], pv_sb[:], ident_t[:])
                V16 = p16.tile([128, 128], F16, name="V16")
                nc.scalar.activation(V16[:], vtp[:].bitcast(F32), AF.Copy,
                                     scale=1.0 / 64.0)

                otn16 = []
                for h in range(QH):
                    stp = psB.tile([128, 128], F32, tag="tmp", name="pstp")
                    nc.tensor.matmul(stp[:], KT16[:], QT16[h][:],
                                     start=True, stop=True)
                    pt16 = p16.tile([128, 128], F16, name=f"ppt{h}")
                    nc.scalar.activation(pt16[:], stp[:], AF.Exp,
                                         scale=EXP_SCALE)
                    nc.vector.tensor_mul(pt16[:], pt16[:],
                                         mask0_t[:, 0:128])
                    ot = psA.tile([128, 128], F32, tag="acc", name="pot")
                    nc.tensor.matmul(ot[:], V16[:], pt16[:], start=True,
                                     stop=True)
                    ds = psB.tile([1, 128], F32, tag="tmp", name="pds")
                    nc.tensor.matmul(ds[:], onr16_t[:], pt16[:], start=True,
                                     stop=True)
                    rc = rcp.tile([1, 128], F32R, tag="rc", name="prc")
                    with nc.allow_low_precision(reason="fp22 softmax recip"):
                        nc.vector.reciprocal(rc[:], ds[:])
                    bc = psB.tile([128, 128], F32, tag="tmp", name="pbc")
                    nc.tensor.matmul(bc[:], onb1_t[:], rc[:], start=True,
                                     stop=True)
                    bcs = pst.tile([128, 128], F32, tag="pst", name="pbcs")
                    nc.scalar.activation(bcs[:], bc[:], AF.Copy)
                    o16 = p16.tile([128, 128], F16, name=f"potn{h}")
                    nc.vector.tensor_mul(o16[:], ot[:], bcs[:])
                    otn16.append(o16)

                for g in range(2):
                    w16o = []
                    for hh in range(QH):
                        wt = pwo16.tile([128, 2048], F16, tag="pwo",
                                        name=f"w16o{hh}")
                        nc.gpsimd.dma_start(
                            wt[:], wo16_d[:, hh, 2048 * g:2048 * (g + 1)])
                        w16o.append(wt)
                    ost = ostp.tile([128, 2048], BF16, tag="ost", name="post")
                    for q in range(4):
                        dc = 4 * g + q
                        acc = psA.tile([128, 512], F32, tag="acc",
                                       name="pacc")
                        for h in range(QH):
                            nc.tensor.matmul(
                                acc[:], otn16[h][:],
                                w16o[h][:, 512 * q:512 * (q + 1)],
                                start=(h == 0), stop=(h == QH - 1))
                        dstc = ost[:, 512 * q:512 * (q + 1)]
                        if q % 2 == 0:
                            nc.scalar.activation(dstc, acc[:], AF.Copy)
                        else:
                            nc.vector.tensor_copy(dstc, acc[:])
                    eng = nc.sync if g == 0 else nc.scalar
                    eng.dma_start(outp_d[g], ost[:])

            patch()

    nc.compile()
    return nc


def _host_tables():
    pos = np.arange(S, dtype=np.float64)
    inv_freq = ROPE_BASE ** (-np.arange(0, HD, 2, dtype=np.float64) / HD)
    ang = np.outer(pos, inv_freq)  # [S, HD/2]
    cos = np.cos(ang).T.astype(np.float32)  # [HD/2, S]
    sin = np.sin(ang).T.astype(np.float32)
    cos2 = np.repeat(cos, 2, axis=0)  # [HD, S]
    sin2 = np.repeat(sin, 2, axis=0)
    sin2[0::2, :] *= -1.0  # even rows get -sin, odd rows +sin
    cos2 *= 1.0 / 64.0
    sin2 *= 1.0 / 64.0

    perm = np.zeros((HD, HD), dtype=np.float32)
    for i in range(HD):
        perm[i ^ 1, i] = 1.0

    jr = np.arange(128)[:, None]
    ir = np.arange(512)[None, :]
    mask0 = np.where(jr <= ir, 1.0, 0.0).astype(np.float32)

    negI = (np.eye(128) * -1e30).astype(np.float32)
    sl = np.zeros((4, 128, 512), dtype=np.float32)
    for m in range(4):
        sl[m] = (jr + 128 * m > ir).astype(np.float32)

    return cos2, sin2, perm, mask0, negI, sl


def kernel(x, Wq, Wk, Wv, Wo):
    global LAST_EXEC_NS
    if "nc" not in _CACHE:
        _CACHE["nc"] = _build_nc()
    nc = _CACHE["nc"]

    x = np.asarray(x, dtype=np.float32).reshape(S, D)
    Wq = np.asarray(Wq, dtype=np.float32)
    Wk = np.asarray(Wk, dtype=np.float32)
    Wv = np.asarray(Wv, dtype=np.float32)
    Wo = np.asarray(Wo, dtype=np.float32)

    xT = np.ascontiguousarray(x.T)  # [D, S]
    x8 = xT.astype(FP8NP)           # [D, S] fp8
    x8r = x8.reshape(DP * 2, 128, S)
    cos2, sin2, perm, mask0, negI, sl = _host_tables()
    cos2c = np.ascontiguousarray(cos2.reshape(HD, SC, 512).transpose(1, 0, 2))
    sin2c = np.ascontiguousarray(sin2.reshape(HD, SC, 512).transpose(1, 0, 2))
    ident = np.eye(HD, dtype=np.float32)
    ones_red = np.ones((128, 1), dtype=np.float32)
    onb16 = np.full((1, 128), 16.0, dtype=np.float32)
    onb1 = np.ones((1, 128), dtype=np.float32)
    # patch x: [128, 32, 128] -> [128, 4096]
    x16 = np.ascontiguousarray(
        xT.reshape(32, 128, S)[:, :, 0:128].transpose(1, 0, 2)
        .reshape(128, 4096)).astype(np.float16)

    in_maps = []
    for c in range(NCORES):
        qs = slice(QH * HD * c, QH * HD * (c + 1))
        ks = slice(HD * c, HD * (c + 1))
        wk8 = (Wk[:, ks] * 64.0).astype(FP8NP).reshape(DP * 2, 128, 128)
        wv8 = (Wv[:, ks] * 64.0).astype(FP8NP).reshape(DP * 2, 128, 128)
        # xw8[sc, dg, p, :]: [xA 512|xB 512|kA|kB|vA|vB] x 2 pair-groups
        xw = np.empty((SC, DG, 128, 3072), dtype=FP8NP)
        for sc in range(SC):
            cs = slice(512 * sc, 512 * (sc + 1))
            for g in range(2):
                base = 1536 * g
                d0 = 4 * np.arange(DG)[:, None] + 2 * g  # [DG,1] d-tile A idx
                xw[sc, :, :, base + 0:base + 512] = x8r[4 * np.arange(DG) + 2 * g, :, cs.start:cs.stop]
                xw[sc, :, :, base + 512:base + 1024] = x8r[4 * np.arange(DG) + 2 * g + 1, :, cs.start:cs.stop]
                xw[sc, :, :, base + 1024:base + 1152] = wk8[4 * np.arange(DG) + 2 * g]
                xw[sc, :, :, base + 1152:base + 1280] = wk8[4 * np.arange(DG) + 2 * g + 1]
                xw[sc, :, :, base + 1280:base + 1408] = wv8[4 * np.arange(DG) + 2 * g]
                xw[sc, :, :, base + 1408:base + 1536] = wv8[4 * np.arange(DG) + 2 * g + 1]

        wq8f = (Wq[:, qs] * 64.0).astype(FP8NP).reshape(DP * 2, 128, 512)
        # wq8[dp] = [tileA 512 | tileB 512]
        wq8 = np.concatenate([wq8f[0::2], wq8f[1::2]], axis=2)

        # wo8[dc, hp, p, :] = [head 2hp rows for dout chunk dc | head 2hp+1]
        # rhs [128, 2, 512]: partition p = hd within head, dim1 = head of pair
        wo64 = (Wo[qs, :] * 64.0).astype(FP8NP)  # [512, 4096]
        wo8 = np.ascontiguousarray(
            wo64.reshape(2, 2, 128, 8, 512)      # [hp, hl, p, dc, cols]
            .transpose(3, 0, 2, 1, 4)            # [dc, hp, p, hl, cols]
            .reshape(8, 2, 128, 1024))

        wqkv16 = np.concatenate(
            [Wq[:, qs] * 64.0, Wk[:, ks] * 64.0, Wv[:, ks] * 64.0],
            axis=1).astype(np.float16)           # [D, 768]
        wqkv16 = np.ascontiguousarray(
            wqkv16.reshape(32, 128, 768).transpose(1, 0, 2))  # [128,32,768]
        wo16 = np.ascontiguousarray(
            Wo[qs, :].reshape(4, 128, 4096).transpose(1, 0, 2)
        ).astype(np.float16)                     # [128, 4, 4096]

        in_maps.append({
            "xw8": xw,
            "wq8": np.ascontiguousarray(wq8),
            "wo8": wo8,
            "cos2": cos2c,
            "sin2": sin2c,
            "perm": perm,
            "masks": mask0,
            "negI": negI,
            "sl": sl,
            "on16": np.full((128, 256), 1.0 / 16.0, dtype=FP8NP),
            "ones_red": ones_red,
            "onb16": onb16,
            "onb1": onb1,
            "onr16": ones_red.astype(np.float16),
            "ident": ident,
            "x16": x16,
            "wqkv16": wqkv16,
            "wo16": wo16,
        })

    res = run_bass_kernel_spmd(nc, in_maps, list(range(NCORES)),
                               trace=TRACE)
    LAST_EXEC_NS = res.exec_time_ns

    acc = res.results[0]["out"].astype(np.float32)
    accp = res.results[0]["outp"].astype(np.float32)
    for c in range(1, NCORES):
        acc = acc + res.results[c]["out"].astype(np.float32)
        accp = accp + res.results[c]["outp"].astype(np.float32)
    # out[dc, c, p, 512*kk + col] -> out[512c + 128kk + p, 512dc + col]
    out = (acc.reshape(8, 4, 128, 4, 512)     # [dc, c, p, kk, col]
           .transpose(1, 3, 2, 0, 4)           # [c, kk, p, dc, col]
           .reshape(S, D))
    out = np.ascontiguousarray(out)
    # patch rows 0..127: outp[g, p, 512q + col] -> [p, 512*(4g+q) + col]
    patch = accp.reshape(2, 128, 4, 512).transpose(1, 0, 2, 3).reshape(128, D)
    out[0:128, :] = patch
    return out.reshape(1, S, D)


# revision 17
# speedup vs baseline: 1.0385x; 1.0385x over previous
"""Tensor-parallel causal GQA self-attention (B=1, S=2048, D=4096, 32 q heads /
8 kv heads, HD=128, interleaved RoPE) on 8 trn2 NeuronCores.

Sharding: core c owns kv head c and q heads 4c..4c+3 (column-parallel
Wq/Wk/Wv, row-parallel Wo).  Each core computes a full [S, D] partial of the
output projection; the host sums the 8 partials (the "all-reduce").

Speed strategy (v2): fp8e4 (e4m3) matmuls with MatmulPerfMode.DoubleRow pack
TWO k-tiles per pass at 0.5 cycles/row -- 4x the fp32r rate -- for the QKV
projections and o_proj.  Attention core (scores/AV) stays fp32r for now.
fp8's ~2.4% RMS quantization error is fine for most rows (softmax averaging
scales output magnitude ~1/sqrt(row)), but rows 0..127 -- which dominate
the output absmax -- are recomputed in an fp16 "patch" pass and overwritten
host-side.

Scaling scheme (keeps fp8 operands ~N(0,1)):
  Wq8/Wk8/Wv8 = fp8(W * 64)      -> q,k,v come out of PSUM x64
  rope cos/sin tables x(1/64)    -> QT/KT at true scale
  V copy applies scale 1/64      -> V at true scale
  softmax 1/sqrt(HD) applied as the Exp activation's scale argument
  otn8 = 16/denom * OT (fp8)     -> Wo8 = fp8(Wo * 64); out copies x(1/1024)
  patch path: Wqkv16 = W*64 (same rope/V handling), Wo16 unscaled.
"""

import sys

if "/opt/trn_rl_repo" not in sys.path:
    sys.path.insert(0, "/opt/trn_rl_repo")

import numpy as np
import ml_dtypes

import concourse.bass as bass
import concourse.tile as tile
from concourse import bacc, mybir
from concourse.bass_utils import run_bass_kernel_spmd

S, D, NH, NKV, HD = 2048, 4096, 32, 8, 128
NCORES = 8
QH = NH // NCORES  # 4 q heads per core
ROPE_BASE = 500000.0

F32 = mybir.dt.float32
F32R = mybir.dt.float32r
F16 = mybir.dt.float16
BF16 = mybir.dt.bfloat16
F8 = mybir.dt.float8e4
AF = mybir.ActivationFunctionType
DR = mybir.MatmulPerfMode.DoubleRow
FP8NP = ml_dtypes.float8_e4m3
BF16NP = ml_dtypes.bfloat16

SC = S // 512   # 4 s-chunks of 512
DP = D // 256   # 16 d-tile pairs
DG = DP // 2    # 8 dma groups (two pairs per DMA)
JT = S // 128   # 16 j-tiles of 128

EXP_SCALE = float(1.0 / np.sqrt(HD))
EXP_BIAS = float(-np.log(16.0))  # keep exp() outputs in fp8 range
OSC = 1.0 / 1024.0  # undo otn x16 and Wo x64

_CACHE = {}

# set by test harness to collect an exec-time profile
TRACE = False
LAST_EXEC_NS = None


def _build_nc():
    nc = bacc.Bacc("TRN2", target_bir_lowering=False, debug=False,
                   num_devices=NCORES)

    xw_d = nc.declare_dram_parameter("xw8", [SC, DG, 128, 3072], F8,
                                     isOutput=False)
    wq_d = nc.declare_dram_parameter("wq8", [DP, 128, 1024], F8,
                                     isOutput=False)
    wo_d = nc.declare_dram_parameter("wo8", [8, 2, 128, 1024], F8,
                                     isOutput=False)
    cos_d = nc.declare_dram_parameter("cos2", [SC, HD, 512], F32, isOutput=False)
    sin_d = nc.declare_dram_parameter("sin2", [SC, HD, 512], F32, isOutput=False)
    perm_d = nc.declare_dram_parameter("perm", [HD, HD], F32R, isOutput=False)
    masks_d = nc.declare_dram_parameter("masks", [128, 512], F32,
                                        isOutput=False)
    negI_d = nc.declare_dram_parameter("negI", [128, 128], F32R,
                                       isOutput=False)
    sl_d = nc.declare_dram_parameter("sl", [4, 128, 512], F32R,
                                     isOutput=False)
    on16_d = nc.declare_dram_parameter("on16", [128, 256], F8,
                                       isOutput=False)
    onr_d = nc.declare_dram_parameter("ones_red", [128, 1], F32R,
                                      isOutput=False)
    onb_d = nc.declare_dram_parameter("onb16", [1, 128], F32R,
                                      isOutput=False)
    onb1_d = nc.declare_dram_parameter("onb1", [1, 128], F32R,
                                       isOutput=False)
    onr16_d = nc.declare_dram_parameter("onr16", [128, 1], F16,
                                        isOutput=False)
    ident_d = nc.declare_dram_parameter("ident", [HD, HD], F32R,
                                        isOutput=False)
    x16_d = nc.declare_dram_parameter("x16", [128, 4096], F16, isOutput=False)
    wqkv16_d = nc.declare_dram_parameter("wqkv16", [128, 32, 768], F16,
                                         isOutput=False)
    wo16_d = nc.declare_dram_parameter("wo16", [128, 4, 4096], F16,
                                       isOutput=False)
    out_d = nc.declare_dram_parameter("out", [8, 4, 128, 2048], BF16,
                                      isOutput=True)
    outp_d = nc.declare_dram_parameter("outp", [2, 128, 2048], BF16,
                                       isOutput=True)

    with tile.TileContext(nc) as tc:
        from contextlib import ExitStack
        ctx = ExitStack()
        with ctx:
            wpool = ctx.enter_context(tc.tile_pool(name="wpool", bufs=16))
            wopool = ctx.enter_context(tc.tile_pool(name="wopool", bufs=16))
            xpool = ctx.enter_context(tc.tile_pool(name="xpool", bufs=3))
            qtp = ctx.enter_context(tc.tile_pool(name="qtp", bufs=9))
            otnp = ctx.enter_context(tc.tile_pool(name="otnp", bufs=8))
            tabp = ctx.enter_context(tc.tile_pool(name="tabp", bufs=4))
            ktp = ctx.enter_context(tc.tile_pool(name="ktp", bufs=4))
            vnp = ctx.enter_context(tc.tile_pool(name="vnp", bufs=4))
            stg = ctx.enter_context(tc.tile_pool(name="stg", bufs=4))
            rawp = ctx.enter_context(tc.tile_pool(name="rawp", bufs=6))
            ptp = ctx.enter_context(tc.tile_pool(name="ptp", bufs=4))
            mkp = ctx.enter_context(tc.tile_pool(name="mkp", bufs=4))
            cst = ctx.enter_context(tc.tile_pool(name="cst", bufs=1))
            rcp = ctx.enter_context(tc.tile_pool(name="rcp", bufs=1))
            ostp = ctx.enter_context(tc.tile_pool(name="ostp", bufs=3))
            # patch pools
            p16 = ctx.enter_context(tc.tile_pool(name="p16", bufs=1))
            pw16 = ctx.enter_context(tc.tile_pool(name="pw16", bufs=16))
            pwo16 = ctx.enter_context(tc.tile_pool(name="pwo16", bufs=6))
            pst = ctx.enter_context(tc.tile_pool(name="pst", bufs=4))
            psA = ctx.enter_context(
                tc.tile_pool(name="psA", bufs=6, space=bass.MemorySpace.PSUM))
            psB = ctx.enter_context(
                tc.tile_pool(name="psB", bufs=2, space=bass.MemorySpace.PSUM))

            # small tables, spread DMAs across both issue paths
            perm_t = cst.tile([HD, HD], F32R, name="perm_t")
            ident_t = cst.tile([HD, HD], F32R, name="ident_t")
            onr_t = cst.tile([128, 1], F32R, name="onr_t")
            onb_t = cst.tile([1, 128], F32R, name="onb_t")
            onb1_t = cst.tile([1, 128], F32R, name="onb1_t")
            onr16_t = cst.tile([128, 1], F16, name="onr16_t")
            mask0_t = mkp.tile([128, 512], F32, name="mask0_t")
            negI_t = mkp.tile([128, 128], F32R, name="negI_t")
            sl_t = [mkp.tile([128, 512], F32R, tag="sl", name=f"sl_{m}")
                    for m in range(4)]
            on16_t = cst.tile([128, 256], F8, name="on16_t")
            ebias_t = cst.tile([128, 1], F32, name="ebias_t")
            nc.gpsimd.memset(ebias_t[:], EXP_BIAS)

            def table_loads():
                yield lambda: nc.sync.dma_start(perm_t[:], perm_d[:])
                yield lambda: nc.scalar.dma_start(ident_t[:], ident_d[:])
                yield lambda: nc.sync.dma_start(onr_t[:], onr_d[:])
                yield lambda: nc.scalar.dma_start(onb_t[:], onb_d[:])
                yield lambda: nc.scalar.dma_start(onb1_t[:], onb1_d[:])
                yield lambda: nc.sync.dma_start(onr16_t[:], onr16_d[:])
                yield lambda: nc.scalar.dma_start(mask0_t[:], masks_d[:])
                yield lambda: nc.sync.dma_start(negI_t[:], negI_d[:])
                yield lambda: nc.scalar.dma_start(on16_t[:], on16_d[:])
                for m in range(4):
                    eng = nc.sync if m % 2 == 0 else nc.scalar
                    yield lambda m=m, eng=eng: eng.dma_start(
                        sl_t[m][:], sl_d[m])

            wq_tiles = [None] * DP

            # persistent activations, one tile per (tensor, s-chunk)
            QTc = [[qtp.tile([HD, 512], F32R, tag="qtc", name=f"qt{h}_{c}")
                    for c in range(SC)] for h in range(QH)]
            KTc = [ktp.tile([HD, 512], F32R, tag="ktc", name=f"kt{c}")
                   for c in range(SC)]
            Vnc = [vnp.tile([128, 512], F8, tag="vnc", name=f"vn{c}")
                   for c in range(SC)]
            # otn8[c][hp]: [128, 1024] fp8 = [hd, (head 2*hp) 512 i | (2*hp+1)]
            otn8 = [[otnp.tile([128, 1024], F8, tag="otn", name=f"otn{c}_{p}")
                     for p in range(2)] for c in range(SC)]

            # ---- phase 1: QKV projections + RoPE + V transpose ----
            def rope_copy(acc_ps, eng):
                raw = rawp.tile([128, 512], F32R, tag="raw", name="rope_raw")
                if eng == "act":
                    nc.scalar.activation(raw[:], acc_ps[:], AF.Copy)
                else:
                    nc.vector.tensor_copy(raw[:], acc_ps[:])
                return raw

            def rope_combine(raw, dest, cc, sn):
                rot = psB.tile([128, 512], F32, tag="tmp", name="rope_rot")
                nc.tensor.matmul(rot[:], perm_t[:], raw[:], start=True,
                                 stop=True)
                t1 = stg.tile([128, 512], F32, tag="stg", name="rope_t1")
                nc.gpsimd.tensor_mul(t1[:], raw[:].bitcast(F32), cc[:])
                t2 = stg.tile([128, 512], F32, tag="stg", name="rope_t2")
                nc.vector.tensor_mul(t2[:], rot[:], sn[:])
                nc.gpsimd.tensor_add(dest[:], t1[:], t2[:])

            boundary_pe = [None]

            def emit_boundary_pe():
                if boundary_pe[0] is not None:
                    boundary_pe[0]()
                    boundary_pe[0] = None

            cs_tiles = [None] * SC  # (cos, sin) chunk tiles, single-use

            # ---- phase 2: attention, interleaved with QKV by chunk ----
            tails = []

            def make_tail(c, h, ot, dsum_bc):
                def tail():
                    rcb = stg.tile([128, 512], F32, tag="stg", name="rcb")
                    with nc.allow_low_precision(reason="fp22 softmax recip"):
                        nc.vector.reciprocal(rcb[:], dsum_bc[:])
                    dst = otn8[c][h // 2][:, 512 * (h % 2):512 * (h % 2 + 1)]
                    nc.vector.tensor_mul(dst, ot[:], rcb[:])
                return tail

            wo_tiles = []

            def o_proj_chunk(c):
                late = c == SC - 1
                for dc in range(8):
                    woa, wob = wo_tiles[dc]
                    ost = ostp.tile([128, 2048], BF16, tag="ost",
                                    name="ost")
                    for kk in range(4):
                        acc = psA.tile([128, 512], F32, tag="acc",
                                       name="oacc")
                        for hp in range(2):
                            lhs = otn8[c][hp][:].rearrange(
                                "p (a b) -> p a b",
                                b=512)[:, :, 128 * kk:128 * (kk + 1)]
                            rhs = woa if hp == 0 else wob
                            nc.tensor.matmul(acc[:], lhs, rhs[:],
                                             start=(hp == 0),
                                             stop=(hp == 1),
                                             perf_mode=DR)
                        dstc = ost[:, 512 * kk:512 * (kk + 1)]
                        act_take = kk == 0 or (late and kk == 1)
                        if act_take:
                            nc.scalar.mul(dstc, acc[:], OSC)
                        else:
                            nc.vector.tensor_scalar_mul(dstc, acc[:], OSC)
                    eng = nc.sync if dc % 2 == 0 else nc.scalar
                    eng.dma_start(out_d[dc, c], ost[:])

            def attn_chunk(c):
                for h in range(QH):
                    qch = QTc[h][c][:]
                    ot = psA.tile([128, 512], F32, tag="acc", name="ot_ps")
                    dsum_bc = psB.tile([128, 512], F32, tag="tmp",
                                       name="dsum_bc")
                    npair = 2 * c + 2
                    pts = [None] * npair

                    def score_pair(t, c=c, qch=qch):
                        pt8 = ptp.tile([128, 1024], F8, tag="pt", name="pt8")
                        for u in range(2):
                            jt = 2 * t + u
                            stp = psA.tile([128, 512], F32, tag="acc",
                                           name="stp")
                            m = jt - 4 * c
                            nc.tensor.matmul(
                                stp[:],
                                KTc[jt // 4][:, 128 * (jt % 4):
                                             128 * (jt % 4 + 1)],
                                qch, start=True, stop=(m < 0))
                            if m >= 0:
                                # additive -1e30 causal mask in PSUM
                                nc.tensor.matmul(stp[:], negI_t[:],
                                                 sl_t[m][:],
                                                 start=False, stop=True)
                            nc.scalar.activation(
                                pt8[:, 512 * u:512 * (u + 1)], stp[:],
                                AF.Exp, scale=EXP_SCALE, bias=ebias_t[:])
                        return pt8

                    def accum_pair(t, pt8, ot=ot, dsum_bc=dsum_bc,
                                   npair=npair):
                        v4 = Vnc[t // 2][:].rearrange("p (a b) -> p a b",
                                                      b=128)
                        pr = pt8[:].rearrange("p (a b) -> p a b", b=512)
                        nc.tensor.matmul(
                            ot[:], v4[:, 2 * (t % 2):2 * (t % 2) + 2, :],
                            pr, start=(t == 0), stop=(t == npair - 1),
                            perf_mode=DR)
                        # all-(1/16) weights: every output row accumulates
                        # sum_j P/256 -> denominator pre-broadcast to all
                        # 128 partitions (x16 otn scale folded in)
                        nc.tensor.matmul(
                            dsum_bc[:],
                            on16_t[:].rearrange("p (a b) -> p a b", b=128),
                            pr, start=(t == 0), stop=(t == npair - 1),
                            perf_mode=DR)

                    for t in range(npair):
                        pts[t] = score_pair(t)
                        if t >= 1:
                            accum_pair(t - 1, pts[t - 1])
                    accum_pair(npair - 1, pts[npair - 1])
                    tails.append(make_tail(c, h, ot, dsum_bc))
                    if len(tails) > 1:
                        tails.pop(0)()
                while tails:
                    tails.pop(0)()
                o_proj_chunk(c)

            pload = {}

            def patch_loads():
                x16t = p16.tile([128, 4096], F16, name="x16t")
                nc.gpsimd.dma_start(x16t[:], x16_d[:])
                pcc = p16.tile([128, 128], F32, name="pcc")
                nc.gpsimd.dma_start(pcc[:], cos_d[0, :, 0:128])
                psn = p16.tile([128, 128], F32, name="psn")
                nc.gpsimd.dma_start(psn[:], sin_d[0, :, 0:128])
                wts = []
                for kt in range(32):
                    wt16 = pw16.tile([128, 768], F16, tag="pw",
                                     name="wt16")
                    nc.gpsimd.dma_start(wt16[:], wqkv16_d[:, kt, :])
                    wts.append(wt16)
                pload.update(x16t=x16t, pcc=pcc, psn=psn, wts=wts)

            for sc in range(SC):
                qps = [psA.tile([128, 512], F32, tag="acc", name=f"qps{h}")
                       for h in range(QH)]
                kps = psA.tile([128, 512], F32, tag="acc", name="kps")
                vps = psA.tile([128, 512], F32, tag="acc", name="vps")
                for dg in range(DG):
                    if sc == 0:
                        for half in range(2):
                            wt = wpool.tile([128, 2, 512], F8, tag="w",
                                            name=f"wq{2 * dg + half}")
                            nc.scalar.dma_start(wt[:], wq_d[2 * dg + half])
                            wq_tiles[2 * dg + half] = wt
                    xt = xpool.tile([128, 3072], F8, tag="x", name="xt")
                    nc.sync.dma_start(xt[:], xw_d[sc, dg])
                    if sc == 0:
                        if dg == 0:
                            _tl = table_loads()
                        next(_tl, lambda: None)()
                        next(_tl, lambda: None)()
                    if dg == 1:
                        emit_boundary_pe()
                    if dg == 4:
                        # prefetch this boundary's cos/sin chunk
                        cc = tabp.tile([128, 512], F32, tag="tab", name="cc")
                        nc.sync.dma_start(cc[:], cos_d[sc])
                        sn = tabp.tile([128, 512], F32, tag="tab", name="sn")
                        nc.sync.dma_start(sn[:], sin_d[sc])
                        cs_tiles[sc] = (cc, sn)
                    x6 = xt[:].rearrange("p (a b) -> p a b", b=512)
                    x24 = xt[:].rearrange("p (a b) -> p a b", b=128)
                    for g in range(2):
                        dp = 2 * dg + g
                        xv = x6[:, 3 * g:3 * g + 2, :]
                        kw = x24[:, 12 * g + 8:12 * g + 10, :]
                        vw = x24[:, 12 * g + 10:12 * g + 12, :]
                        st = dp == 0
                        sp = dp == DP - 1
                        wqt = wq_tiles[dp]
                        for h in range(QH):
                            nc.tensor.matmul(
                                qps[h][:], wqt[:, :, HD * h:HD * (h + 1)],
                                xv, start=st, stop=sp, perf_mode=DR)
                        nc.tensor.matmul(kps[:], kw, xv, start=st, stop=sp,
                                         perf_mode=DR)
                        nc.tensor.matmul(vps[:], vw, xv, start=st, stop=sp,
                                         perf_mode=DR)

                raw_k = rope_copy(kps, "act")
                vt_sb = rope_copy(vps, "dve")
                raw_q = [None] * QH
                raw_q[0] = rope_copy(qps[0], "dve")

                def boundary(sc=sc, raw_k=raw_k, vt_sb=vt_sb, raw_q=raw_q):
                    cc, sn = cs_tiles[sc]
                    rope_combine(raw_k, KTc[sc], cc, sn)
                    rope_combine(raw_q[0], QTc[0][sc], cc, sn)
                    for k4 in range(4):
                        vtp = psB.tile([128, 128], F32R, tag="tmp",
                                       name="vtp")
                        nc.tensor.transpose(
                            vtp[:], vt_sb[:, 128 * k4:128 * (k4 + 1)],
                            ident_t[:])
                        nc.scalar.activation(
                            Vnc[sc][:, 128 * k4:128 * (k4 + 1)],
                            vtp[:].bitcast(F32), AF.Copy, scale=1.0 / 64.0)
                    for h in range(1, QH):
                        rope_combine(raw_q[h], QTc[h][sc], cc, sn)

                boundary_pe[0] = boundary
                if sc == 1:
                    for dc in range(8):
                        woa = wopool.tile([128, 2, 512], F8, tag="wo",
                                          name=f"woa{dc}")
                        nc.sync.dma_start(woa[:], wo_d[dc, 0])
                        wob = wopool.tile([128, 2, 512], F8, tag="wo",
                                          name=f"wob{dc}")
                        nc.scalar.dma_start(wob[:], wo_d[dc, 1])
                        wo_tiles.append((woa, wob))
                # drain all projection accumulators before attention so
                # psA slots are free for score/AV/o_proj rotation
                for h in range(1, QH):
                    raw_q[h] = rope_copy(qps[h],
                                         "act" if h % 2 == 0 else "dve")
                if sc >= 1:
                    attn_chunk(sc - 1)
                if sc == SC - 1:
                    emit_boundary_pe()
                    attn_chunk(SC - 1)
                    patch_loads()

            # ---- phase 4: fp16 patch for output rows 0..127 ----
            def patch():
                x16t, pcc, psn = pload["x16t"], pload["pcc"], pload["psn"]
                pq = [psA.tile([128, 128], F32, tag="acc", name=f"pq{h}")
                      for h in range(QH)]
                pk = psA.tile([128, 128], F32, tag="acc", name="pk")
                pv = psA.tile([128, 128], F32, tag="acc", name="pv")
                for kt in range(32):
                    wt16 = pload["wts"][kt]
                    rhs = x16t[:, 128 * kt:128 * (kt + 1)]
                    st = kt == 0
                    sp = kt == 31
                    for h in range(QH):
                        nc.tensor.matmul(pq[h][:],
                                         wt16[:, 128 * h:128 * (h + 1)],
                                         rhs, start=st, stop=sp)
                    nc.tensor.matmul(pk[:], wt16[:, 512:640], rhs,
                                     start=st, stop=sp)
                    nc.tensor.matmul(pv[:], wt16[:, 640:768], rhs,
                                     start=st, stop=sp)

                # patch rope (fp32 math, fp16 outputs)
                def prope(acc_ps, nm):
                    raw = pst.tile([128, 128], F32R, tag="pst",
                                   name=f"raw_{nm}")
                    nc.scalar.activation(raw[:], acc_ps[:], AF.Copy)
                    rot = psB.tile([128, 128], F32, tag="tmp", name="prot")
                    nc.tensor.matmul(rot[:], perm_t[:], raw[:], start=True,
                                     stop=True)
                    t1 = pst.tile([128, 128], F32, tag="pst", name="pt1")
                    nc.vector.tensor_mul(t1[:], raw[:].bitcast(F32), pcc[:])
                    t2 = pst.tile([128, 128], F32, tag="pst", name="pt2")
                    nc.vector.tensor_mul(t2[:], rot[:], psn[:])
                    d16 = p16.tile([128, 128], F16, name=f"d16_{nm}")
                    nc.vector.tensor_add(d16[:], t1[:], t2[:])
                    return d16

                KT16 = prope(pk, "k")
                QT16 = [prope(pq[h], f"q{h}") for h in range(QH)]
                pv_sb = pst.tile([128, 128], F32R, tag="pst", name="pv_sb")
                nc.vector.tensor_copy(pv_sb[:], pv[:])
                vtp = psB.tile([128, 128], F32R, tag="tmp", name="pvtp")
                nc.tensor.transpose(vtp[:], pv_sb[:], ident_t[:])
                V16 = p16.tile([128, 128], F16, name="V16")
                nc.scalar.activation(V16[:], vtp[:].bitcast(F32), AF.Copy,
                                     scale=1.0 / 64.0)

                otn16 = []
                for h in range(QH):
                    stp = psB.tile([128, 128], F32, tag="tmp", name="pstp")
                    nc.tensor.matmul(stp[:], KT16[:], QT16[h][:],
                                     start=True, stop=True)
                    pt16 = p16.tile([128, 128], F16, name=f"ppt{h}")
                    nc.scalar.activation(pt16[:], stp[:], AF.Exp,
                                         scale=EXP_SCALE)
                    nc.vector.tensor_mul(pt16[:], pt16[:],
                                         mask0_t[:, 0:128])
                    ot = psA.tile([128, 128], F32, tag="acc", name="pot")
                    nc.tensor.matmul(ot[:], V16[:], pt16[:], start=True,
                                     stop=True)
                    ds = psB.tile([1, 128], F32, tag="tmp", name="pds")
                    nc.tensor.matmul(ds[:], onr16_t[:], pt16[:], start=True,
                                     stop=True)
                    rc = rcp.tile([1, 128], F32R, tag="rc", name="prc")
                    with nc.allow_low_precision(reason="fp22 softmax recip"):
                        nc.vector.reciprocal(rc[:], ds[:])
                    bc = psB.tile([128, 128], F32, tag="tmp", name="pbc")
                    nc.tensor.matmul(bc[:], onb1_t[:], rc[:], start=True,
                                     stop=True)
                    bcs = pst.tile([128, 128], F32, tag="pst", name="pbcs")
                    nc.scalar.activation(bcs[:], bc[:], AF.Copy)
                    o16 = p16.tile([128, 128], F16, name=f"potn{h}")
                    nc.vector.tensor_mul(o16[:], ot[:], bcs[:])
                    otn16.append(o16)

                for g in range(2):
                    w16o = []
                    for hh in range(QH):
                        wt = pwo16.tile([128, 2048], F16, tag="pwo",
                                        name=f"w16o{hh}")
                        nc.gpsimd.dma_start(
                            wt[:], wo16_d[:, hh, 2048 * g:2048 * (g + 1)])
                        w16o.append(wt)
                    ost = ostp.tile([128, 2048], BF16, tag="ost", name="post")
                    for q in range(4):
                        dc = 4 * g + q
                        acc = psA.tile([128, 512], F32, tag="acc",
                                       name="pacc")
                        for h in range(QH):
                            nc.tensor.matmul(
                                acc[:], otn16[h][:],
                                w16o[h][:, 512 * q:512 * (q + 1)],
                                start=(h == 0), stop=(h == QH - 1))
                        dstc = ost[:, 512 * q:512 * (q + 1)]
                        if q % 2 == 0:
                            nc.scalar.activation(dstc, acc[:], AF.Copy)
                        else:
                            nc.vector.tensor_copy(dstc, acc[:])
                    eng = nc.sync if g == 0 else nc.scalar
                    eng.dma_start(outp_d[g], ost[:])

            patch()

    nc.compile()
    return nc


def _host_tables():
    pos = np.arange(S, dtype=np.float64)
    inv_freq = ROPE_BASE ** (-np.arange(0, HD, 2, dtype=np.float64) / HD)
    ang = np.outer(pos, inv_freq)  # [S, HD/2]
    cos = np.cos(ang).T.astype(np.float32)  # [HD/2, S]
    sin = np.sin(ang).T.astype(np.float32)
    cos2 = np.repeat(cos, 2, axis=0)  # [HD, S]
    sin2 = np.repeat(sin, 2, axis=0)
    sin2[0::2, :] *= -1.0  # even rows get -sin, odd rows +sin
    cos2 *= 1.0 / 64.0
    sin2 *= 1.0 / 64.0

    perm = np.zeros((HD, HD), dtype=np.float32)
    for i in range(HD):
        perm[i ^ 1, i] = 1.0

    jr = np.arange(128)[:, None]
    ir = np.arange(512)[None, :]
    mask0 = np.where(jr <= ir, 1.0, 0.0).astype(np.float32)

    negI = (np.eye(128) * -1e30).astype(np.float32)
    sl = np.zeros((4, 128, 512), dtype=np.float32)
    for m in range(4):
        sl[m] = (jr + 128 * m > ir).astype(np.float32)

    return cos2, sin2, perm, mask0, negI, sl


def kernel(x, Wq, Wk, Wv, Wo):
    global LAST_EXEC_NS
    if "nc" not in _CACHE:
        _CACHE["nc"] = _build_nc()
    nc = _CACHE["nc"]

    x = np.asarray(x, dtype=np.float32).reshape(S, D)
    Wq = np.asarray(Wq, dtype=np.float32)
    Wk = np.asarray(Wk, dtype=np.float32)
    Wv = np.asarray(Wv, dtype=np.float32)
    Wo = np.asarray(Wo, dtype=np.float32)

    xT = np.ascontiguousarray(x.T)  # [D, S]
    x8 = xT.astype(FP8NP)           # [D, S] fp8
    x8r = x8.reshape(DP * 2, 128, S)
    cos2, sin2, perm, mask0, negI, sl = _host_tables()
    cos2c = np.ascontiguousarray(cos2.reshape(HD, SC, 512).transpose(1, 0, 2))
    sin2c = np.ascontiguousarray(sin2.reshape(HD, SC, 512).transpose(1, 0, 2))
    ident = np.eye(HD, dtype=np.float32)
    ones_red = np.ones((128, 1), dtype=np.float32)
    onb16 = np.full((1, 128), 16.0, dtype=np.float32)
    onb1 = np.ones((1, 128), dtype=np.float32)
    # patch x: [128, 32, 128] -> [128, 4096]
    x16 = np.ascontiguousarray(
        xT.reshape(32, 128, S)[:, :, 0:128].transpose(1, 0, 2)
        .reshape(128, 4096)).astype(np.float16)

    in_maps = []
    for c in range(NCORES):
        qs = slice(QH * HD * c, QH * HD * (c + 1))
        ks = slice(HD * c, HD * (c + 1))
        wk8 = (Wk[:, ks] * 64.0).astype(FP8NP).reshape(DP * 2, 128, 128)
        wv8 = (Wv[:, ks] * 64.0).astype(FP8NP).reshape(DP * 2, 128, 128)
        # xw8[sc, dg, p, :]: [xA 512|xB 512|kA|kB|vA|vB] x 2 pair-groups
        xw = np.empty((SC, DG, 128, 3072), dtype=FP8NP)
        for sc in range(SC):
            cs = slice(512 * sc, 512 * (sc + 1))
            for g in range(2):
                base = 1536 * g
                d0 = 4 * np.arange(DG)[:, None] + 2 * g  # [DG,1] d-tile A idx
                xw[sc, :, :, base + 0:base + 512] = x8r[4 * np.arange(DG) + 2 * g, :, cs.start:cs.stop]
                xw[sc, :, :, base + 512:base + 1024] = x8r[4 * np.arange(DG) + 2 * g + 1, :, cs.start:cs.stop]
                xw[sc, :, :, base + 1024:base + 1152] = wk8[4 * np.arange(DG) + 2 * g]
                xw[sc, :, :, base + 1152:base + 1280] = wk8[4 * np.arange(DG) + 2 * g + 1]
                xw[sc, :, :, base + 1280:base + 1408] = wv8[4 * np.arange(DG) + 2 * g]
                xw[sc, :, :, base + 1408:base + 1536] = wv8[4 * np.arange(DG) + 2 * g + 1]

        wq8f = (Wq[:, qs] * 64.0).astype(FP8NP).reshape(DP * 2, 128, 512)
        # wq8[dp] = [tileA 512 | tileB 512]
        wq8 = np.concatenate([wq8f[0::2], wq8f[1::2]], axis=2)

        # wo8[dc, hp, p, :] = [head 2hp rows for dout chunk dc | head 2hp+1]
        # rhs [128, 2, 512]: partition p = hd within head, dim1 = head of pair
        wo64 = (Wo[qs, :] * 64.0).astype(FP8NP)  # [512, 4096]
        wo8 = np.ascontiguousarray(
            wo64.reshape(2, 2, 128, 8, 512)      # [hp, hl, p, dc, cols]
            .transpose(3, 0, 2, 1, 4)            # [dc, hp, p, hl, cols]
            .reshape(8, 2, 128, 1024))

        wqkv16 = np.concatenate(
            [Wq[:, qs] * 64.0, Wk[:, ks] * 64.0, Wv[:, ks] * 64.0],
            axis=1).astype(np.float16)           # [D, 768]
        wqkv16 = np.ascontiguousarray(
            wqkv16.reshape(32, 128, 768).transpose(1, 0, 2))  # [128,32,768]
        wo16 = np.ascontiguousarray(
            Wo[qs, :].reshape(4, 128, 4096).transpose(1, 0, 2)
        ).astype(np.float16)                     # [128, 4, 4096]

        in_maps.append({
            "xw8": xw,
            "wq8": np.ascontiguousarray(wq8),
            "wo8": wo8,
            "cos2": cos2c,
            "sin2": sin2c,
            "perm": perm,
            "masks": mask0,
            "negI": negI,
            "sl": sl,
            "on16": np.full((128, 256), 1.0 / 16.0, dtype=FP8NP),
            "ones_red": ones_red,
            "onb16": onb16,
            "onb1": onb1,
            "onr16": ones_red.astype(np.float16),
            "ident": ident,
            "x16": x16,
            "wqkv16": wqkv16,
            "wo16": wo16,
        })

    res = run_bass_kernel_spmd(nc, in_maps, list(range(NCORES)),
                               trace=TRACE)
    LAST_EXEC_NS = res.exec_time_ns

    acc = res.results[0]["out"].astype(np.float32)
    accp = res.results[0]["outp"].astype(np.float32)
    for c in range(1, NCORES):
        acc = acc + res.results[c]["out"].astype(np.float32)
        accp = accp + res.results[c]["outp"].astype(np.float32)
    # out[dc, c, p, 512*kk + col] -> out[512c + 128kk + p, 512dc + col]
    out = (acc.reshape(8, 4, 128, 4, 512)     # [dc, c, p, kk, col]
           .transpose(1, 3, 2, 0, 4)           # [c, kk, p, dc, col]
           .reshape(S, D))
    out = np.ascontiguousarray(out)
    # patch rows 0..127: outp[g, p, 512q + col] -> [p, 512*(4g+q) + col]
    patch = accp.reshape(2, 128, 4, 512).transpose(1, 0, 2, 3).reshape(128, D)
    out[0:128, :] = patch
    return out.reshape(1, S, D)


# revision 18
# speedup vs baseline: 1.0713x; 1.0316x over previous
"""Tensor-parallel causal GQA self-attention (B=1, S=2048, D=4096, 32 q heads /
8 kv heads, HD=128, interleaved RoPE) on 8 trn2 NeuronCores.

Sharding: core c owns kv head c and q heads 4c..4c+3 (column-parallel
Wq/Wk/Wv, row-parallel Wo).  Each core computes a full [S, D] partial of the
output projection; the host sums the 8 partials (the "all-reduce").

Speed strategy (v2): fp8e4 (e4m3) matmuls with MatmulPerfMode.DoubleRow pack
TWO k-tiles per pass at 0.5 cycles/row -- 4x the fp32r rate -- for the QKV
projections and o_proj.  Attention core (scores/AV) stays fp32r for now.
fp8's ~2.4% RMS quantization error is fine for most rows (softmax averaging
scales output magnitude ~1/sqrt(row)), but rows 0..127 -- which dominate
the output absmax -- are recomputed in an fp16 "patch" pass and overwritten
host-side.

Scaling scheme (keeps fp8 operands ~N(0,1)):
  Wq8/Wk8/Wv8 = fp8(W * 64)      -> q,k,v come out of PSUM x64
  rope cos/sin tables x(1/64)    -> QT/KT at true scale
  V copy applies scale 1/64      -> V at true scale
  softmax 1/sqrt(HD) applied as the Exp activation's scale argument
  otn8 = 16/denom * OT (fp8)     -> Wo8 = fp8(Wo * 64); out copies x(1/1024)
  patch path: Wqkv16 = W*64 (same rope/V handling), Wo16 unscaled.
"""

import sys

if "/opt/trn_rl_repo" not in sys.path:
    sys.path.insert(0, "/opt/trn_rl_repo")

import numpy as np
import ml_dtypes

import concourse.bass as bass
import concourse.tile as tile
from concourse import bacc, mybir
from concourse.bass_utils import run_bass_kernel_spmd

S, D, NH, NKV, HD = 2048, 4096, 32, 8, 128
NCORES = 8
QH = NH // NCORES  # 4 q heads per core
ROPE_BASE = 500000.0

F32 = mybir.dt.float32
F32R = mybir.dt.float32r
F16 = mybir.dt.float16
BF16 = mybir.dt.bfloat16
F8 = mybir.dt.float8e4
AF = mybir.ActivationFunctionType
DR = mybir.MatmulPerfMode.DoubleRow
FP8NP = ml_dtypes.float8_e4m3
BF16NP = ml_dtypes.bfloat16

SC = S // 512   # 4 s-chunks of 512
DP = D // 256   # 16 d-tile pairs
DG = DP // 2    # 8 dma groups (two pairs per DMA)
JT = S // 128   # 16 j-tiles of 128

EXP_SCALE = float(1.0 / np.sqrt(HD))
EXP_BIAS = float(-np.log(16.0))  # keep exp() outputs in fp8 range
OSC = 1.0 / 1024.0  # undo otn x16 and Wo x64

_CACHE = {}

# set by test harness to collect an exec-time profile
TRACE = False
LAST_EXEC_NS = None


def _build_nc():
    nc = bacc.Bacc("TRN2", target_bir_lowering=False, debug=False,
                   num_devices=NCORES)

    xw_d = nc.declare_dram_parameter("xw8", [SC, DG, 128, 3072], F8,
                                     isOutput=False)
    wq_d = nc.declare_dram_parameter("wq8", [DP, 128, 1024], F8,
                                     isOutput=False)
    wo_d = nc.declare_dram_parameter("wo8", [8, 2, 128, 1024], F8,
                                     isOutput=False)
    cos_d = nc.declare_dram_parameter("cos2", [SC, HD, 512], F32, isOutput=False)
    sin_d = nc.declare_dram_parameter("sin2", [SC, HD, 512], F32, isOutput=False)
    perm_d = nc.declare_dram_parameter("perm", [HD, HD], F32R, isOutput=False)
    masks_d = nc.declare_dram_parameter("masks", [128, 512], F32,
                                        isOutput=False)
    negI_d = nc.declare_dram_parameter("negI", [128, 128], F32R,
                                       isOutput=False)
    sl_d = nc.declare_dram_parameter("sl", [4, 128, 512], F32R,
                                     isOutput=False)
    on16_d = nc.declare_dram_parameter("on16", [128, 256], F8,
                                       isOutput=False)
    onr_d = nc.declare_dram_parameter("ones_red", [128, 1], F32R,
                                      isOutput=False)
    onb_d = nc.declare_dram_parameter("onb16", [1, 128], F32R,
                                      isOutput=False)
    onb1_d = nc.declare_dram_parameter("onb1", [1, 128], F32R,
                                       isOutput=False)
    onr16_d = nc.declare_dram_parameter("onr16", [128, 1], F16,
                                        isOutput=False)
    ident_d = nc.declare_dram_parameter("ident", [HD, HD], F32R,
                                        isOutput=False)
    x16_d = nc.declare_dram_parameter("x16", [128, 4096], F16, isOutput=False)
    wqkv16_d = nc.declare_dram_parameter("wqkv16", [128, 32, 768], F16,
                                         isOutput=False)
    wo16_d = nc.declare_dram_parameter("wo16", [128, 4, 4096], F16,
                                       isOutput=False)
    out_d = nc.declare_dram_parameter("out", [8, 4, 128, 2048], BF16,
                                      isOutput=True)
    outp_d = nc.declare_dram_parameter("outp", [2, 128, 2048], BF16,
                                       isOutput=True)

    with tile.TileContext(nc) as tc:
        from contextlib import ExitStack
        ctx = ExitStack()
        with ctx:
            wpool = ctx.enter_context(tc.tile_pool(name="wpool", bufs=16))
            wopool = ctx.enter_context(tc.tile_pool(name="wopool", bufs=16))
            xpool = ctx.enter_context(tc.tile_pool(name="xpool", bufs=3))
            qtp = ctx.enter_context(tc.tile_pool(name="qtp", bufs=9))
            otnp = ctx.enter_context(tc.tile_pool(name="otnp", bufs=8))
            tabp = ctx.enter_context(tc.tile_pool(name="tabp", bufs=4))
            ktp = ctx.enter_context(tc.tile_pool(name="ktp", bufs=4))
            vnp = ctx.enter_context(tc.tile_pool(name="vnp", bufs=4))
            stg = ctx.enter_context(tc.tile_pool(name="stg", bufs=4))
            rawp = ctx.enter_context(tc.tile_pool(name="rawp", bufs=6))
            ptp = ctx.enter_context(tc.tile_pool(name="ptp", bufs=4))
            mkp = ctx.enter_context(tc.tile_pool(name="mkp", bufs=4))
            cst = ctx.enter_context(tc.tile_pool(name="cst", bufs=1))
            rcp = ctx.enter_context(tc.tile_pool(name="rcp", bufs=1))
            ostp = ctx.enter_context(tc.tile_pool(name="ostp", bufs=3))
            # patch pools
            p16 = ctx.enter_context(tc.tile_pool(name="p16", bufs=1))
            pw16 = ctx.enter_context(tc.tile_pool(name="pw16", bufs=16))
            pwo16 = ctx.enter_context(tc.tile_pool(name="pwo16", bufs=6))
            pst = ctx.enter_context(tc.tile_pool(name="pst", bufs=4))
            psA = ctx.enter_context(
                tc.tile_pool(name="psA", bufs=6, space=bass.MemorySpace.PSUM))
            psB = ctx.enter_context(
                tc.tile_pool(name="psB", bufs=2, space=bass.MemorySpace.PSUM))

            # small tables, spread DMAs across both issue paths
            perm_t = cst.tile([HD, HD], F32R, name="perm_t")
            ident_t = cst.tile([HD, HD], F32R, name="ident_t")
            onr_t = cst.tile([128, 1], F32R, name="onr_t")
            onb_t = cst.tile([1, 128], F32R, name="onb_t")
            onb1_t = cst.tile([1, 128], F32R, name="onb1_t")
            onr16_t = cst.tile([128, 1], F16, name="onr16_t")
            mask0_t = mkp.tile([128, 512], F32, name="mask0_t")
            negI_t = mkp.tile([128, 128], F32R, name="negI_t")
            sl_t = [mkp.tile([128, 512], F32R, tag="sl", name=f"sl_{m}")
                    for m in range(4)]
            on16_t = cst.tile([128, 256], F8, name="on16_t")
            ebias_t = cst.tile([128, 1], F32, name="ebias_t")
            nc.gpsimd.memset(ebias_t[:], EXP_BIAS)

            def table_loads():
                yield lambda: nc.sync.dma_start(perm_t[:], perm_d[:])
                yield lambda: nc.scalar.dma_start(ident_t[:], ident_d[:])
                yield lambda: nc.sync.dma_start(onr_t[:], onr_d[:])
                yield lambda: nc.scalar.dma_start(onb_t[:], onb_d[:])
                yield lambda: nc.scalar.dma_start(onb1_t[:], onb1_d[:])
                yield lambda: nc.sync.dma_start(onr16_t[:], onr16_d[:])
                yield lambda: nc.scalar.dma_start(mask0_t[:], masks_d[:])
                yield lambda: nc.sync.dma_start(negI_t[:], negI_d[:])
                yield lambda: nc.scalar.dma_start(on16_t[:], on16_d[:])
                for m in range(4):
                    eng = nc.sync if m % 2 == 0 else nc.scalar
                    yield lambda m=m, eng=eng: eng.dma_start(
                        sl_t[m][:], sl_d[m])

            wq_tiles = [None] * DP

            # persistent activations, one tile per (tensor, s-chunk)
            QTc = [[qtp.tile([HD, 512], F32R, tag="qtc", name=f"qt{h}_{c}")
                    for c in range(SC)] for h in range(QH)]
            KTc = [ktp.tile([HD, 512], F32R, tag="ktc", name=f"kt{c}")
                   for c in range(SC)]
            Vnc = [vnp.tile([128, 512], F8, tag="vnc", name=f"vn{c}")
                   for c in range(SC)]
            # otn8[c][hp]: [128, 1024] fp8 = [hd, (head 2*hp) 512 i | (2*hp+1)]
            otn8 = [[otnp.tile([128, 1024], F8, tag="otn", name=f"otn{c}_{p}")
                     for p in range(2)] for c in range(SC)]

            # ---- phase 1: QKV projections + RoPE + V transpose ----
            def rope_copy(acc_ps, eng):
                raw = rawp.tile([128, 512], F32R, tag="raw", name="rope_raw")
                if eng == "act":
                    nc.scalar.activation(raw[:], acc_ps[:], AF.Copy)
                else:
                    nc.vector.tensor_copy(raw[:], acc_ps[:])
                return raw

            def rope_combine(raw, dest, cc, sn):
                rot = psB.tile([128, 512], F32, tag="tmp", name="rope_rot")
                nc.tensor.matmul(rot[:], perm_t[:], raw[:], start=True,
                                 stop=True)
                t1 = stg.tile([128, 512], F32, tag="stg", name="rope_t1")
                nc.gpsimd.tensor_mul(t1[:], raw[:].bitcast(F32), cc[:])
                t2 = stg.tile([128, 512], F32, tag="stg", name="rope_t2")
                nc.vector.tensor_mul(t2[:], rot[:], sn[:])
                nc.gpsimd.tensor_add(dest[:], t1[:], t2[:])

            boundary_pe = [None]

            def emit_boundary_pe():
                if boundary_pe[0] is not None:
                    boundary_pe[0]()
                    boundary_pe[0] = None

            cs_tiles = [None] * SC  # (cos, sin) chunk tiles, single-use

            # ---- phase 2: attention, interleaved with QKV by chunk ----
            tails = []

            def make_tail(c, h, ot, dsum_bc):
                def tail():
                    rcb = stg.tile([128, 512], F32, tag="stg", name="rcb")
                    with nc.allow_low_precision(reason="fp22 softmax recip"):
                        nc.vector.reciprocal(rcb[:], dsum_bc[:])
                    dst = otn8[c][h // 2][:, 512 * (h % 2):512 * (h % 2 + 1)]
                    nc.vector.tensor_mul(dst, ot[:], rcb[:])
                return tail

            wo_tiles = []

            def o_proj_chunk(c):
                late = c == SC - 1
                for dc in range(8):
                    woa, wob = wo_tiles[dc]
                    ost = ostp.tile([128, 2048], BF16, tag="ost",
                                    name="ost")
                    for kk in range(4):
                        acc = psA.tile([128, 512], F32, tag="acc",
                                       name="oacc")
                        for hp in range(2):
                            lhs = otn8[c][hp][:].rearrange(
                                "p (a b) -> p a b",
                                b=512)[:, :, 128 * kk:128 * (kk + 1)]
                            rhs = woa if hp == 0 else wob
                            nc.tensor.matmul(acc[:], lhs, rhs[:],
                                             start=(hp == 0),
                                             stop=(hp == 1),
                                             perf_mode=DR)
                        dstc = ost[:, 512 * kk:512 * (kk + 1)]
                        act_take = kk == 0 or (late and kk == 1)
                        if act_take:
                            nc.scalar.mul(dstc, acc[:], OSC)
                        else:
                            nc.vector.tensor_scalar_mul(dstc, acc[:], OSC)
                    eng = nc.sync if dc % 2 == 0 else nc.scalar
                    eng.dma_start(out_d[dc, c], ost[:])

            def attn_chunk(c):
                for h in range(QH):
                    qch = QTc[h][c][:]
                    ot = psA.tile([128, 512], F32, tag="acc", name="ot_ps")
                    dsum_bc = psB.tile([128, 512], F32, tag="tmp",
                                       name="dsum_bc")
                    npair = 2 * c + 2
                    pts = [None] * npair

                    def score_pair(t, c=c, qch=qch):
                        pt8 = ptp.tile([128, 1024], F8, tag="pt", name="pt8")
                        for u in range(2):
                            jt = 2 * t + u
                            stp = psA.tile([128, 512], F32, tag="acc",
                                           name="stp")
                            m = jt - 4 * c
                            nc.tensor.matmul(
                                stp[:],
                                KTc[jt // 4][:, 128 * (jt % 4):
                                             128 * (jt % 4 + 1)],
                                qch, start=True, stop=(m < 0))
                            if m >= 0:
                                # additive -1e30 causal mask in PSUM
                                nc.tensor.matmul(stp[:], negI_t[:],
                                                 sl_t[m][:],
                                                 start=False, stop=True)
                            nc.scalar.activation(
                                pt8[:, 512 * u:512 * (u + 1)], stp[:],
                                AF.Exp, scale=EXP_SCALE, bias=ebias_t[:])
                        return pt8

                    def accum_pair(t, pt8, ot=ot, dsum_bc=dsum_bc,
                                   npair=npair):
                        v4 = Vnc[t // 2][:].rearrange("p (a b) -> p a b",
                                                      b=128)
                        pr = pt8[:].rearrange("p (a b) -> p a b", b=512)
                        nc.tensor.matmul(
                            ot[:], v4[:, 2 * (t % 2):2 * (t % 2) + 2, :],
                            pr, start=(t == 0), stop=(t == npair - 1),
                            perf_mode=DR)
                        # all-(1/16) weights: every output row accumulates
                        # sum_j P/256 -> denominator pre-broadcast to all
                        # 128 partitions (x16 otn scale folded in)
                        nc.tensor.matmul(
                            dsum_bc[:],
                            on16_t[:].rearrange("p (a b) -> p a b", b=128),
                            pr, start=(t == 0), stop=(t == npair - 1),
                            perf_mode=DR)

                    for t in range(npair):
                        pts[t] = score_pair(t)
                        if t >= 1:
                            accum_pair(t - 1, pts[t - 1])
                    accum_pair(npair - 1, pts[npair - 1])
                    tails.append(make_tail(c, h, ot, dsum_bc))
                    if len(tails) > 1:
                        tails.pop(0)()
                while tails:
                    tails.pop(0)()
                o_proj_chunk(c)

            pload = {}

            def patch_loads():
                x16t = p16.tile([128, 4096], F16, name="x16t")
                nc.sync.dma_start(x16t[:], x16_d[:])
                pcc = p16.tile([128, 128], F32, name="pcc")
                nc.scalar.dma_start(pcc[:], cos_d[0, :, 0:128])
                psn = p16.tile([128, 128], F32, name="psn")
                nc.scalar.dma_start(psn[:], sin_d[0, :, 0:128])
                wts = []
                for kt in range(32):
                    wt16 = pw16.tile([128, 768], F16, tag="pw",
                                     name="wt16")
                    # tiles 0-15 on sync never block; 16+ may wait on a
                    # pool slot, so keep them off the sync queue
                    eng = nc.sync if kt < 16 else nc.scalar
                    eng.dma_start(wt16[:], wqkv16_d[:, kt, :])
                    wts.append(wt16)
                pload.update(x16t=x16t, pcc=pcc, psn=psn, wts=wts)

            for sc in range(SC):
                qps = [psA.tile([128, 512], F32, tag="acc", name=f"qps{h}")
                       for h in range(QH)]
                kps = psA.tile([128, 512], F32, tag="acc", name="kps")
                vps = psA.tile([128, 512], F32, tag="acc", name="vps")
                for dg in range(DG):
                    if sc == 0:
                        for half in range(2):
                            wt = wpool.tile([128, 2, 512], F8, tag="w",
                                            name=f"wq{2 * dg + half}")
                            nc.scalar.dma_start(wt[:], wq_d[2 * dg + half])
                            wq_tiles[2 * dg + half] = wt
                    xt = xpool.tile([128, 3072], F8, tag="x", name="xt")
                    nc.sync.dma_start(xt[:], xw_d[sc, dg])
                    if sc == 0:
                        if dg == 0:
                            _tl = table_loads()
                        next(_tl, lambda: None)()
                        next(_tl, lambda: None)()
                    if dg == 1:
                        emit_boundary_pe()
                    if dg == 4:
                        # prefetch this boundary's cos/sin chunk
                        cc = tabp.tile([128, 512], F32, tag="tab", name="cc")
                        nc.sync.dma_start(cc[:], cos_d[sc])
                        sn = tabp.tile([128, 512], F32, tag="tab", name="sn")
                        nc.sync.dma_start(sn[:], sin_d[sc])
                        cs_tiles[sc] = (cc, sn)
                    x6 = xt[:].rearrange("p (a b) -> p a b", b=512)
                    x24 = xt[:].rearrange("p (a b) -> p a b", b=128)
                    for g in range(2):
                        dp = 2 * dg + g
                        xv = x6[:, 3 * g:3 * g + 2, :]
                        kw = x24[:, 12 * g + 8:12 * g + 10, :]
                        vw = x24[:, 12 * g + 10:12 * g + 12, :]
                        st = dp == 0
                        sp = dp == DP - 1
                        wqt = wq_tiles[dp]
                        for h in range(QH):
                            nc.tensor.matmul(
                                qps[h][:], wqt[:, :, HD * h:HD * (h + 1)],
                                xv, start=st, stop=sp, perf_mode=DR)
                        nc.tensor.matmul(kps[:], kw, xv, start=st, stop=sp,
                                         perf_mode=DR)
                        nc.tensor.matmul(vps[:], vw, xv, start=st, stop=sp,
                                         perf_mode=DR)

                raw_k = rope_copy(kps, "act")
                vt_sb = rope_copy(vps, "dve")
                raw_q = [None] * QH
                raw_q[0] = rope_copy(qps[0], "dve")

                def boundary(sc=sc, raw_k=raw_k, vt_sb=vt_sb, raw_q=raw_q):
                    cc, sn = cs_tiles[sc]
                    rope_combine(raw_k, KTc[sc], cc, sn)
                    rope_combine(raw_q[0], QTc[0][sc], cc, sn)
                    for k4 in range(4):
                        vtp = psB.tile([128, 128], F32R, tag="tmp",
                                       name="vtp")
                        nc.tensor.transpose(
                            vtp[:], vt_sb[:, 128 * k4:128 * (k4 + 1)],
                            ident_t[:])
                        nc.scalar.activation(
                            Vnc[sc][:, 128 * k4:128 * (k4 + 1)],
                            vtp[:].bitcast(F32), AF.Copy, scale=1.0 / 64.0)
                    for h in range(1, QH):
                        rope_combine(raw_q[h], QTc[h][sc], cc, sn)

                boundary_pe[0] = boundary
                if sc == 1:
                    for dc in range(8):
                        woa = wopool.tile([128, 2, 512], F8, tag="wo",
                                          name=f"woa{dc}")
                        nc.sync.dma_start(woa[:], wo_d[dc, 0])
                        wob = wopool.tile([128, 2, 512], F8, tag="wo",
                                          name=f"wob{dc}")
                        nc.scalar.dma_start(wob[:], wo_d[dc, 1])
                        wo_tiles.append((woa, wob))
                # drain all projection accumulators before attention so
                # psA slots are free for score/AV/o_proj rotation
                for h in range(1, QH):
                    raw_q[h] = rope_copy(qps[h],
                                         "act" if h % 2 == 0 else "dve")
                if sc >= 1:
                    attn_chunk(sc - 1)
                if sc == SC - 1:
                    emit_boundary_pe()
                    attn_chunk(SC - 1)
                    patch_loads()

            # ---- phase 4: fp16 patch for output rows 0..127 ----
            def patch():
                x16t, pcc, psn = pload["x16t"], pload["pcc"], pload["psn"]
                pq = [psA.tile([128, 128], F32, tag="acc", name=f"pq{h}")
                      for h in range(QH)]
                pk = psA.tile([128, 128], F32, tag="acc", name="pk")
                pv = psA.tile([128, 128], F32, tag="acc", name="pv")
                for kt in range(32):
                    wt16 = pload["wts"][kt]
                    rhs = x16t[:, 128 * kt:128 * (kt + 1)]
                    st = kt == 0
                    sp = kt == 31
                    for h in range(QH):
                        nc.tensor.matmul(pq[h][:],
                                         wt16[:, 128 * h:128 * (h + 1)],
                                         rhs, start=st, stop=sp)
                    nc.tensor.matmul(pk[:], wt16[:, 512:640], rhs,
                                     start=st, stop=sp)
                    nc.tensor.matmul(pv[:], wt16[:, 640:768], rhs,
                                     start=st, stop=sp)

                # patch rope (fp32 math, fp16 outputs)
                def prope(acc_ps, nm):
                    raw = pst.tile([128, 128], F32R, tag="pst",
                                   name=f"raw_{nm}")
                    nc.scalar.activation(raw[:], acc_ps[:], AF.Copy)
                    rot = psB.tile([128, 128], F32, tag="tmp", name="prot")
                    nc.tensor.matmul(rot[:], perm_t[:], raw[:], start=True,
                                     stop=True)
                    t1 = pst.tile([128, 128], F32, tag="pst", name="pt1")
                    nc.vector.tensor_mul(t1[:], raw[:].bitcast(F32), pcc[:])
                    t2 = pst.tile([128, 128], F32, tag="pst", name="pt2")
                    nc.vector.tensor_mul(t2[:], rot[:], psn[:])
                    d16 = p16.tile([128, 128], F16, name=f"d16_{nm}")
                    nc.vector.tensor_add(d16[:], t1[:], t2[:])
                    return d16

                KT16 = prope(pk, "k")
                QT16 = [prope(pq[h], f"q{h}") for h in range(QH)]
                pv_sb = pst.tile([128, 128], F32R, tag="pst", name="pv_sb")
                nc.vector.tensor_copy(pv_sb[:], pv[:])
                vtp = psB.tile([128, 128], F32R, tag="tmp", name="pvtp")
                nc.tensor.transpose(vtp[:], pv_sb[:], ident_t[:])
                V16 = p16.tile([128, 128], F16, name="V16")
                nc.scalar.activation(V16[:], vtp[:].bitcast(F32), AF.Copy,
                                     scale=1.0 / 64.0)

                otn16 = []
                for h in range(QH):
                    stp = psB.tile([128, 128], F32, tag="tmp", name="pstp")
                    nc.tensor.matmul(stp[:], KT16[:], QT16[h][:],
                                     start=True, stop=True)
                    pt16 = p16.tile([128, 128], F16, name=f"ppt{h}")
                    nc.scalar.activation(pt16[:], stp[:], AF.Exp,
                                         scale=EXP_SCALE)
                    nc.vector.tensor_mul(pt16[:], pt16[:],
                                         mask0_t[:, 0:128])
                    ot = psA.tile([128, 128], F32, tag="acc", name="pot")
                    nc.tensor.matmul(ot[:], V16[:], pt16[:], start=True,
                                     stop=True)
                    ds = psB.tile([1, 128], F32, tag="tmp", name="pds")
                    nc.tensor.matmul(ds[:], onr16_t[:], pt16[:], start=True,
                                     stop=True)
                    rc = rcp.tile([1, 128], F32R, tag="rc", name="prc")
                    with nc.allow_low_precision(reason="fp22 softmax recip"):
                        nc.vector.reciprocal(rc[:], ds[:])
                    bc = psB.tile([128, 128], F32, tag="tmp", name="pbc")
                    nc.tensor.matmul(bc[:], onb1_t[:], rc[:], start=True,
                                     stop=True)
                    bcs = pst.tile([128, 128], F32, tag="pst", name="pbcs")
                    nc.scalar.activation(bcs[:], bc[:], AF.Copy)
                    o16 = p16.tile([128, 128], F16, name=f"potn{h}")
                    nc.vector.tensor_mul(o16[:], ot[:], bcs[:])
                    otn16.append(o16)

                for g in range(2):
                    w16o = []
                    for hh in range(QH):
                        wt = pwo16.tile([128, 2048], F16, tag="pwo",
                                        name=f"w16o{hh}")
                        eng = nc.sync if hh % 2 == 0 else nc.scalar
                        eng.dma_start(
                            wt[:], wo16_d[:, hh, 2048 * g:2048 * (g + 1)])
                        w16o.append(wt)
                    ost = ostp.tile([128, 2048], BF16, tag="ost", name="post")
                    for q in range(4):
                        dc = 4 * g + q
                        acc = psA.tile([128, 512], F32, tag="acc",
                                       name="pacc")
                        for h in range(QH):
                            nc.tensor.matmul(
                                acc[:], otn16[h][:],
                                w16o[h][:, 512 * q:512 * (q + 1)],
                                start=(h == 0), stop=(h == QH - 1))
                        dstc = ost[:, 512 * q:512 * (q + 1)]
                        if q % 2 == 0:
                            nc.scalar.activation(dstc, acc[:], AF.Copy)
                        else:
                            nc.vector.tensor_copy(dstc, acc[:])
                    eng = nc.sync if g == 0 else nc.scalar
                    eng.dma_start(outp_d[g], ost[:])

            patch()

    nc.compile()
    return nc


def _host_tables():
    pos = np.arange(S, dtype=np.float64)
    inv_freq = ROPE_BASE ** (-np.arange(0, HD, 2, dtype=np.float64) / HD)
    ang = np.outer(pos, inv_freq)  # [S, HD/2]
    cos = np.cos(ang).T.astype(np.float32)  # [HD/2, S]
    sin = np.sin(ang).T.astype(np.float32)
    cos2 = np.repeat(cos, 2, axis=0)  # [HD, S]
    sin2 = np.repeat(sin, 2, axis=0)
    sin2[0::2, :] *= -1.0  # even rows get -sin, odd rows +sin
    cos2 *= 1.0 / 64.0
    sin2 *= 1.0 / 64.0

    perm = np.zeros((HD, HD), dtype=np.float32)
    for i in range(HD):
        perm[i ^ 1, i] = 1.0

    jr = np.arange(128)[:, None]
    ir = np.arange(512)[None, :]
    mask0 = np.where(jr <= ir, 1.0, 0.0).astype(np.float32)

    negI = (np.eye(128) * -1e30).astype(np.float32)
    sl = np.zeros((4, 128, 512), dtype=np.float32)
    for m in range(4):
        sl[m] = (jr + 128 * m > ir).astype(np.float32)

    return cos2, sin2, perm, mask0, negI, sl


def kernel(x, Wq, Wk, Wv, Wo):
    global LAST_EXEC_NS
    if "nc" not in _CACHE:
        _CACHE["nc"] = _build_nc()
    nc = _CACHE["nc"]

    x = np.asarray(x, dtype=np.float32).reshape(S, D)
    Wq = np.asarray(Wq, dtype=np.float32)
    Wk = np.asarray(Wk, dtype=np.float32)
    Wv = np.asarray(Wv, dtype=np.float32)
    Wo = np.asarray(Wo, dtype=np.float32)

    xT = np.ascontiguousarray(x.T)  # [D, S]
    x8 = xT.astype(FP8NP)           # [D, S] fp8
    x8r = x8.reshape(DP * 2, 128, S)
    cos2, sin2, perm, mask0, negI, sl = _host_tables()
    cos2c = np.ascontiguousarray(cos2.reshape(HD, SC, 512).transpose(1, 0, 2))
    sin2c = np.ascontiguousarray(sin2.reshape(HD, SC, 512).transpose(1, 0, 2))
    ident = np.eye(HD, dtype=np.float32)
    ones_red = np.ones((128, 1), dtype=np.float32)
    onb16 = np.full((1, 128), 16.0, dtype=np.float32)
    onb1 = np.ones((1, 128), dtype=np.float32)
    # patch x: [128, 32, 128] -> [128, 4096]
    x16 = np.ascontiguousarray(
        xT.reshape(32, 128, S)[:, :, 0:128].transpose(1, 0, 2)
        .reshape(128, 4096)).astype(np.float16)

    in_maps = []
    for c in range(NCORES):
        qs = slice(QH * HD * c, QH * HD * (c + 1))
        ks = slice(HD * c, HD * (c + 1))
        wk8 = (Wk[:, ks] * 64.0).astype(FP8NP).reshape(DP * 2, 128, 128)
        wv8 = (Wv[:, ks] * 64.0).astype(FP8NP).reshape(DP * 2, 128, 128)
        # xw8[sc, dg, p, :]: [xA 512|xB 512|kA|kB|vA|vB] x 2 pair-groups
        xw = np.empty((SC, DG, 128, 3072), dtype=FP8NP)
        for sc in range(SC):
            cs = slice(512 * sc, 512 * (sc + 1))
            for g in range(2):
                base = 1536 * g
                d0 = 4 * np.arange(DG)[:, None] + 2 * g  # [DG,1] d-tile A idx
                xw[sc, :, :, base + 0:base + 512] = x8r[4 * np.arange(DG) + 2 * g, :, cs.start:cs.stop]
                xw[sc, :, :, base + 512:base + 1024] = x8r[4 * np.arange(DG) + 2 * g + 1, :, cs.start:cs.stop]
                xw[sc, :, :, base + 1024:base + 1152] = wk8[4 * np.arange(DG) + 2 * g]
                xw[sc, :, :, base + 1152:base + 1280] = wk8[4 * np.arange(DG) + 2 * g + 1]
                xw[sc, :, :, base + 1280:base + 1408] = wv8[4 * np.arange(DG) + 2 * g]
                xw[sc, :, :, base + 1408:base + 1536] = wv8[4 * np.arange(DG) + 2 * g + 1]

        wq8f = (Wq[:, qs] * 64.0).astype(FP8NP).reshape(DP * 2, 128, 512)
        # wq8[dp] = [tileA 512 | tileB 512]
        wq8 = np.concatenate([wq8f[0::2], wq8f[1::2]], axis=2)

        # wo8[dc, hp, p, :] = [head 2hp rows for dout chunk dc | head 2hp+1]
        # rhs [128, 2, 512]: partition p = hd within head, dim1 = head of pair
        wo64 = (Wo[qs, :] * 64.0).astype(FP8NP)  # [512, 4096]
        wo8 = np.ascontiguousarray(
            wo64.reshape(2, 2, 128, 8, 512)      # [hp, hl, p, dc, cols]
            .transpose(3, 0, 2, 1, 4)            # [dc, hp, p, hl, cols]
            .reshape(8, 2, 128, 1024))

        wqkv16 = np.concatenate(
            [Wq[:, qs] * 64.0, Wk[:, ks] * 64.0, Wv[:, ks] * 64.0],
            axis=1).astype(np.float16)           # [D, 768]
        wqkv16 = np.ascontiguousarray(
            wqkv16.reshape(32, 128, 768).transpose(1, 0, 2))  # [128,32,768]
        wo16 = np.ascontiguousarray(
            Wo[qs, :].reshape(4, 128, 4096).transpose(1, 0, 2)
        ).astype(np.float16)                     # [128, 4, 4096]

        in_maps.append({
            "xw8": xw,
            "wq8": np.ascontiguousarray(wq8),
            "wo8": wo8,
            "cos2": cos2c,
            "sin2": sin2c,
            "perm": perm,
            "masks": mask0,
            "negI": negI,
            "sl": sl,
            "on16": np.full((128, 256), 1.0 / 16.0, dtype=FP8NP),
            "ones_red": ones_red,
            "onb16": onb16,
            "onb1": onb1,
            "onr16": ones_red.astype(np.float16),
            "ident": ident,
            "x16": x16,
            "wqkv16": wqkv16,
            "wo16": wo16,
        })

    res = run_bass_kernel_spmd(nc, in_maps, list(range(NCORES)),
                               trace=TRACE)
    LAST_EXEC_NS = res.exec_time_ns

    acc = res.results[0]["out"].astype(np.float32)
    accp = res.results[0]["outp"].astype(np.float32)
    for c in range(1, NCORES):
        acc = acc + res.results[c]["out"].astype(np.float32)
        accp = accp + res.results[c]["outp"].astype(np.float32)
    # out[dc, c, p, 512*kk + col] -> out[512c + 128kk + p, 512dc + col]
    out = (acc.reshape(8, 4, 128, 4, 512)     # [dc, c, p, kk, col]
           .transpose(1, 3, 2, 0, 4)           # [c, kk, p, dc, col]
           .reshape(S, D))
    out = np.ascontiguousarray(out)
    # patch rows 0..127: outp[g, p, 512q + col] -> [p, 512*(4g+q) + col]
    patch = accp.reshape(2, 128, 4, 512).transpose(1, 0, 2, 3).reshape(128, D)
    out[0:128, :] = patch
    return out.reshape(1, S, D)
